# revision 13
# baseline (speedup 1.0000x reference)
"""Trainium2 Bass kernel for CP-decomposed conv2d (nn_CPDConvolution2D).

Reference computation (NCHW, fp32):
  h = conv1x1(x, W1)         [N,64,224,224] -> [N,32,224,224]
  h = depthwise 3x1 vertical (pad 1)
  h = depthwise 1x3 horizontal (pad 1)
  y = conv1x1(h, W4) + bias  -> [N,128,224,224]

Sharding: data-parallel over batch, 2 images per core on 8 cores.

Per-core layout: images are processed in 7 strips of HB=32 rows.  A
strip's 32 rows are split over 4 "row groups" of GB=8 rows; group j
lives on SBUF/PSUM partitions [32j, 32j+32).  Stage A (1x1, K=64,
M=32) uses PE col-tiling so the 4 groups' outputs fill all 128 PSUM
partitions of one bank; the depthwise taps then run as per-partition
DVE multiply-accumulates (weights are per-partition scalars); stage B
(1x1, K=32, M=128) uses PE row-tiling, each group contracting its own
partition range into its own PSUM bank.  The vertical conv needs one
halo row on each side of a group, so stage A computes GB+2=10 rows per
group (x is loaded with one halo row per strip and zeroed at image
edges, which makes the padding rows fall out automatically).
"""
import os
import sys
import types

sys.path.insert(0, '/opt/trn_rl_repo')

import numpy as np

import concourse.bass as bass
import concourse.mybir as mybir
from concourse.tile import TileContext

# ---------------------------------------------------------------------------
# Environment compat: NTFF profile hook (for trace timing) and a sync
# legalizer for this container's walrus build, which accepts at most one
# sem wait and one sem update per instruction while Tile attaches several
# at dependency joins.
# ---------------------------------------------------------------------------


def _install_ntff_hook():
    if "antenv.axon_hooks" in sys.modules:
        return
    try:
        from trn_agent_boot.trn_boot import _ntff_profile_via_ctypes
    except ImportError:
        return
    _hook = _ntff_profile_via_ctypes('/opt/axon/libaxon_pjrt.so')
    m = types.ModuleType("antenv.axon_hooks")
    m.get_axon_ntff_profile_hook = lambda: _hook
    m.set_axon_ntff_profile_hook = lambda h: None
    sys.modules["antenv.axon_hooks"] = m
    from concourse import bass_utils
    bass_utils.upload_artifacts = lambda tmpdir: "local://" + tmpdir


def _legalize_sync(nc):
    """Split multi-wait/multi-update instructions onto same-engine NoOps.

    Engine queues execute in order, so waits hoisted onto NoOps placed
    before an instruction still gate it; an update pushed onto a NoOp
    after a compute instruction fires only once that instruction has
    completed (the documented-safe `op; nop().then_inc(sem)` idiom).
    Moving a DMA's completion update is NOT safe -- assert instead.
    """
    for f in nc.m.functions:
        for bb in f.blocks:
            idx = 0
            while idx < len(bb.instructions):
                inst = bb.instructions[idx]
                si = inst.sync_info
                if si is None:
                    idx += 1
                    continue
                waits = si.on_wait
                if waits is not None and len(waits) > 1:
                    extra = list(waits[:-1])
                    del si.on_wait[:-1]
                    for w in extra:
                        nop = mybir.InstNoOp(
                            name=nc.get_next_instruction_name(),
                            engine=inst.engine, ins=[], outs=[],
                        )
                        nop.sync_info = mybir.SyncInfo(on_wait=[w], on_update=[])
                        nc.register_instruction(nop)
                        bb.instructions.insert(idx, nop)
                        idx += 1
                    si = inst.sync_info
                upds = si.on_update
                if upds is not None and len(upds) > 1:
                    assert not isinstance(
                        inst,
                        (mybir.InstDMACopy, mybir.InstDMA, mybir.InstDmaTransposeAnt),
                    ), f"multi-update on DMA instruction {inst.name}"
                    extra = list(upds[1:])
                    del si.on_update[1:]
                    for u in extra:
                        nop = mybir.InstNoOp(
                            name=nc.get_next_instruction_name(),
                            engine=inst.engine, ins=[], outs=[],
                        )
                        nop.sync_info = mybir.SyncInfo(on_wait=[], on_update=[u])
                        nc.register_instruction(nop)
                        bb.instructions.insert(idx + 1, nop)
                idx += 1


# ---------------------------------------------------------------------------
# Problem shapes (hardcoded per spec)
# ---------------------------------------------------------------------------
N_FULL, S_CH, H_IMG, W_IMG = 16, 64, 224, 224
R_CH, T_CH = 32, 128
N_CORES = 8
N_PER_CORE = N_FULL // N_CORES     # 2 images per core
HB = 32                            # strip height (rows)
GB = HB // 4                       # rows per partition group
N_STRIPS = H_IMG // HB             # 7
FP32 = mybir.dt.float32
F32R = mybir.dt.float32r
# float32r streams 1 PE column/cycle (vs 4 for fp32's two half-speed
# passes) at TF32-like precision (~1e-4 scale-relative matmul error).
# Walrus only accepts it with dst partition 0, so stage A (col-tiled,
# dst partition 32j) stays fp32 and only stage B (row-tiled, dst 0)
# uses it.
MM_DT = F32R if int(os.environ.get("KERNEL_F32R", "1")) else FP32

_CACHE = {}
LAST_EXEC_TIME_NS = None


def _build_nc():
    nc = bass.Bass(target_bir_lowering=False)

    x = nc.dram_tensor("x", [N_PER_CORE, S_CH, H_IMG, W_IMG], FP32,
                       kind="ExternalInput")
    w1T = nc.dram_tensor("w1T", [S_CH, R_CH], FP32, kind="ExternalInput")
    wv = nc.dram_tensor("wv", [128, 3], FP32, kind="ExternalInput")
    wh = nc.dram_tensor("wh", [128, 3], FP32, kind="ExternalInput")
    w4s = nc.dram_tensor("w4s", [128, 128], MM_DT, kind="ExternalInput")
    bias = nc.dram_tensor("bias", [128, 1], FP32, kind="ExternalInput")
    y = nc.dram_tensor("y", [N_PER_CORE, T_CH, H_IMG, W_IMG], FP32,
                       kind="ExternalOutput")

    with TileContext(nc) as tc:
        with (
            tc.tile_pool(name="consts", bufs=1) as consts,
            tc.tile_pool(name="xin", bufs=2) as xin,
            tc.tile_pool(name="mid", bufs=2) as mid,
            tc.tile_pool(name="oout", bufs=2) as oout,
            tc.tile_pool(name="psA", bufs=2, space="PSUM") as psumA,
            tc.tile_pool(name="psB", bufs=6, space="PSUM") as psumB,
        ):
            w1T_t = consts.tile([S_CH, R_CH], FP32)
            wv_t = consts.tile([128, 3], FP32)
            wh_t = consts.tile([128, 3], FP32)
            w4s_t = consts.tile([128, 128], MM_DT)
            bias_t = consts.tile([128, 1], FP32)
            nc.sync.dma_start(out=w1T_t[:], in_=w1T[:, :])
            nc.sync.dma_start(out=wv_t[:], in_=wv[:, :])
            nc.sync.dma_start(out=wh_t[:], in_=wh[:, :])
            nc.sync.dma_start(out=w4s_t[:], in_=w4s[:, :])
            nc.sync.dma_start(out=bias_t[:], in_=bias[:, :])

            for n in range(N_PER_CORE):
                for s in range(N_STRIPS):
                    h0 = s * HB
                    # ---- load x strip with one halo row each side ----
                    x_t = xin.tile([S_CH, HB + 2, W_IMG], FP32)
                    if s == 0:
                        nc.gpsimd.memset(x_t[:, 0:1, :], 0.0)
                        nc.sync.dma_start(out=x_t[:, 1:HB + 2, :],
                                          in_=x[n, :, 0:HB + 1, :])
                    elif s == N_STRIPS - 1:
                        nc.sync.dma_start(out=x_t[:, 0:HB + 1, :],
                                          in_=x[n, :, h0 - 1:h0 + HB, :])
                        nc.gpsimd.memset(x_t[:, HB + 1:HB + 2, :], 0.0)
                    else:
                        nc.sync.dma_start(out=x_t[:, :, :],
                                          in_=x[n, :, h0 - 1:h0 + HB + 1, :])

                    # ---- stage A: 1x1 S->R, col-tiled x4 ----
                    # h1p[p in grp j, m, :] = h1[row h0 + 8j - 1 + m, :]
                    h1p = mid.tile([128, GB + 2, W_IMG], FP32, tag="h1p")
                    for c in range((GB + 2) // 2):
                        psA = psumA.tile([128, 2, W_IMG], FP32)
                        for j in range(4):
                            r0 = j * GB + 2 * c
                            nc.tensor.matmul(
                                psA[32 * j:32 * j + 32, :, :],
                                w1T_t[:, :],
                                x_t[:, r0:r0 + 2, :],
                                start=True, stop=True,
                                tile_position=(0, 32 * j),
                            )
                        nc.scalar.copy(h1p[:, 2 * c:2 * c + 2, :], psA[:, :, :])

                    # ---- vertical 3x1 depthwise (per-partition scalars) ----
                    h2p = mid.tile([128, GB, W_IMG + 2], FP32, tag="h2p")
                    nc.gpsimd.memset(h2p[:, :, 0:1], 0.0)
                    nc.gpsimd.memset(h2p[:, :, W_IMG + 1:W_IMG + 2], 0.0)
                    h2c = h2p[:, :, 1:W_IMG + 1]
                    # tap 0 on the otherwise-idle GPSIMD engine (single-input
                    # tensor ops run at line rate there), MAC taps on DVE
                    nc.gpsimd.tensor_scalar_mul(
                        h2c, h1p[:, 0:GB, :], wv_t[:, 0:1])
                    for kv in (1, 2):
                        nc.vector.scalar_tensor_tensor(
                            h2c, h1p[:, kv:kv + GB, :], wv_t[:, kv:kv + 1], h2c,
                            op0=mybir.AluOpType.mult, op1=mybir.AluOpType.add)

                    # ---- horizontal 1x3 depthwise ----
                    # accumulate taps 0-1 in fp32, round once into the
                    # MM_DT tile on the final tap
                    h3a = mid.tile([128, GB, W_IMG], FP32, tag="h3a")
                    h3 = mid.tile([128, GB, W_IMG], MM_DT, tag="h3")
                    nc.gpsimd.tensor_scalar_mul(
                        h3a[:, :, :], h2p[:, :, 0:W_IMG], wh_t[:, 0:1])
                    nc.vector.scalar_tensor_tensor(
                        h3a[:, :, :], h2p[:, :, 1:1 + W_IMG],
                        wh_t[:, 1:2], h3a[:, :, :],
                        op0=mybir.AluOpType.mult, op1=mybir.AluOpType.add)
                    nc.vector.scalar_tensor_tensor(
                        h3[:, :, :], h2p[:, :, 2:2 + W_IMG],
                        wh_t[:, 2:3], h3a[:, :, :],
                        op0=mybir.AluOpType.mult, op1=mybir.AluOpType.add)

                    # ---- stage B: 1x1 R->T row-tiled x4, + bias ----
                    o_t = oout.tile([T_CH, HB, W_IMG], FP32)
                    for c in range(GB // 2):
                        for g in range(4):
                            psB = psumB.tile([128, 2, W_IMG], FP32)
                            nc.tensor.matmul(
                                psB[:, :, :],
                                w4s_t[32 * g:32 * g + 32, :],
                                h3[32 * g:32 * g + 32, 2 * c:2 * c + 2, :],
                                start=True, stop=True,
                                tile_position=(32 * g, 0),
                            )
                            orow = g * GB + 2 * c
                            # split bias-copies over ACT and DVE to balance
                            if g == 3:
                                nc.vector.tensor_scalar_add(
                                    o_t[:, orow:orow + 2, :], psB[:, :, :],
                                    bias_t[:, 0:1])
                            else:
                                nc.scalar.add(
                                    o_t[:, orow:orow + 2, :], psB[:, :, :],
                                    bias_t[:, 0:1])

                    # output goes out on the scalar HWDGE ring so reads
                    # (sync ring) and writes overlap instead of FIFO-ing
                    # behind each other on one queue
                    nc.scalar.dma_start(out=y[n, :, h0:h0 + HB, :],
                                        in_=o_t[:, :, :])

    _legalize_sync(nc)
    return nc


def _prep_weights(s_to_r_weight, depth_vert_weight, depth_hor_weight,
                  r_to_t_weight, r_to_t_bias):
    w1T = np.ascontiguousarray(
        s_to_r_weight[:, :, 0, 0].T.astype(np.float32))          # [64, 32]
    wv = np.ascontiguousarray(
        np.tile(depth_vert_weight[:, 0, :, 0], (4, 1)).astype(np.float32))
    wh = np.ascontiguousarray(
        np.tile(depth_hor_weight[:, 0, 0, :], (4, 1)).astype(np.float32))
    w4s = np.ascontiguousarray(
        np.tile(r_to_t_weight[:, :, 0, 0].T, (4, 1)).astype(np.float32))
    b = np.ascontiguousarray(
        r_to_t_bias.reshape(T_CH, 1).astype(np.float32))
    return w1T, wv, wh, w4s, b


def kernel(x, s_to_r_weight, depth_vert_weight, depth_hor_weight,
           r_to_t_weight, r_to_t_bias):
    global LAST_EXEC_TIME_NS
    _install_ntff_hook()
    from concourse.bass_utils import run_bass_kernel_spmd

    if "nc" not in _CACHE:
        _CACHE["nc"] = _build_nc()
    nc = _CACHE["nc"]

    x = np.asarray(x, dtype=np.float32)
    w1T, wv, wh, w4s, b = _prep_weights(
        np.asarray(s_to_r_weight), np.asarray(depth_vert_weight),
        np.asarray(depth_hor_weight), np.asarray(r_to_t_weight),
        np.asarray(r_to_t_bias))

    in_maps = []
    for i in range(N_CORES):
        in_maps.append({
            "x": np.ascontiguousarray(x[i * N_PER_CORE:(i + 1) * N_PER_CORE]),
            "w1T": w1T, "wv": wv, "wh": wh, "w4s": w4s, "bias": b,
        })

    trace = bool(int(os.environ.get("KERNEL_TRACE", "0")))
    res = run_bass_kernel_spmd(nc, in_maps, core_ids=list(range(N_CORES)),
                               trace=trace)
    LAST_EXEC_TIME_NS = res.exec_time_ns

    out = np.empty((N_FULL, T_CH, H_IMG, W_IMG), dtype=np.float32)
    for i in range(N_CORES):
        out[i * N_PER_CORE:(i + 1) * N_PER_CORE] = res.results[i]["y"]
    return out


# revision 15
# speedup vs baseline: 3.0482x; 3.0482x over previous
"""Trainium2 Bass kernel for CP-decomposed conv2d (nn_CPDConvolution2D).

Reference computation (NCHW, fp32):
  h = conv1x1(x, W1)         [N,64,224,224] -> [N,32,224,224]
  h = depthwise 3x1 vertical (pad 1)
  h = depthwise 1x3 horizontal (pad 1)
  y = conv1x1(h, W4) + bias  -> [N,128,224,224]

Sharding: data-parallel over batch, 2 images per core on 8 cores.

Per-core layout: images are processed in 7 strips of HB=32 rows.  A
strip's 32 rows are split over 4 "row groups" of GB=8 rows; group j
lives on SBUF/PSUM partitions [32j, 32j+32).  Stage A (1x1, K=64,
M=32) uses PE col-tiling so the 4 groups' outputs fill all 128 PSUM
partitions of one bank; the depthwise taps then run as per-partition
DVE multiply-accumulates (weights are per-partition scalars); stage B
(1x1, K=32, M=128) uses PE row-tiling, each group contracting its own
partition range into its own PSUM bank.  The vertical conv needs one
halo row on each side of a group, so stage A computes GB+2=10 rows per
group (x is loaded with one halo row per strip and zeroed at image
edges, which makes the padding rows fall out automatically).
"""
import os
import sys
import types

sys.path.insert(0, '/opt/trn_rl_repo')

import numpy as np

import concourse.bass as bass
import concourse.mybir as mybir
from concourse.tile import TileContext

# ---------------------------------------------------------------------------
# Environment compat: NTFF profile hook (for trace timing) and a sync
# legalizer for this container's walrus build, which accepts at most one
# sem wait and one sem update per instruction while Tile attaches several
# at dependency joins.
# ---------------------------------------------------------------------------


def _install_ntff_hook():
    if "antenv.axon_hooks" in sys.modules:
        return
    try:
        from trn_agent_boot.trn_boot import _ntff_profile_via_ctypes
    except ImportError:
        return
    _hook = _ntff_profile_via_ctypes('/opt/axon/libaxon_pjrt.so')
    m = types.ModuleType("antenv.axon_hooks")
    m.get_axon_ntff_profile_hook = lambda: _hook
    m.set_axon_ntff_profile_hook = lambda h: None
    sys.modules["antenv.axon_hooks"] = m
    from concourse import bass_utils
    bass_utils.upload_artifacts = lambda tmpdir: "local://" + tmpdir


def _legalize_sync(nc):
    """Split multi-wait/multi-update instructions onto same-engine NoOps.

    Engine queues execute in order, so waits hoisted onto NoOps placed
    before an instruction still gate it; an update pushed onto a NoOp
    after a compute instruction fires only once that instruction has
    completed (the documented-safe `op; nop().then_inc(sem)` idiom).
    Moving a DMA's completion update is NOT safe -- assert instead.
    """
    for f in nc.m.functions:
        for bb in f.blocks:
            idx = 0
            while idx < len(bb.instructions):
                inst = bb.instructions[idx]
                si = inst.sync_info
                if si is None:
                    idx += 1
                    continue
                waits = si.on_wait
                if waits is not None and len(waits) > 1:
                    extra = list(waits[:-1])
                    del si.on_wait[:-1]
                    for w in extra:
                        nop = mybir.InstNoOp(
                            name=nc.get_next_instruction_name(),
                            engine=inst.engine, ins=[], outs=[],
                        )
                        nop.sync_info = mybir.SyncInfo(on_wait=[w], on_update=[])
                        nc.register_instruction(nop)
                        bb.instructions.insert(idx, nop)
                        idx += 1
                    si = inst.sync_info
                upds = si.on_update
                if upds is not None and len(upds) > 1:
                    assert not isinstance(
                        inst,
                        (mybir.InstDMACopy, mybir.InstDMA, mybir.InstDmaTransposeAnt),
                    ), f"multi-update on DMA instruction {inst.name}"
                    extra = list(upds[1:])
                    del si.on_update[1:]
                    for u in extra:
                        nop = mybir.InstNoOp(
                            name=nc.get_next_instruction_name(),
                            engine=inst.engine, ins=[], outs=[],
                        )
                        nop.sync_info = mybir.SyncInfo(on_wait=[], on_update=[u])
                        nc.register_instruction(nop)
                        bb.instructions.insert(idx + 1, nop)
                idx += 1


# ---------------------------------------------------------------------------
# Problem shapes (hardcoded per spec)
# ---------------------------------------------------------------------------
N_FULL, S_CH, H_IMG, W_IMG = 16, 64, 224, 224
R_CH, T_CH = 32, 128
N_CORES = 8
N_PER_CORE = N_FULL // N_CORES     # 2 images per core
HB = 32                            # strip height (rows)
GB = HB // 4                       # rows per partition group
N_STRIPS = H_IMG // HB             # 7
FP32 = mybir.dt.float32
F32R = mybir.dt.float32r
# float32r streams 1 PE column/cycle (vs 4 for fp32's two half-speed
# passes) at TF32-like precision (~1e-4 scale-relative matmul error).
# Walrus only accepts it with dst partition 0, so stage A (col-tiled,
# dst partition 32j) stays fp32 and only stage B (row-tiled, dst 0)
# uses it.
MM_DT = F32R if int(os.environ.get("KERNEL_F32R", "1")) else FP32

_CACHE = {}
LAST_EXEC_TIME_NS = None


def _build_nc():
    nc = bass.Bass(target_bir_lowering=False)

    x = nc.dram_tensor("x", [N_PER_CORE, S_CH, H_IMG, W_IMG], FP32,
                       kind="ExternalInput")
    w1T = nc.dram_tensor("w1T", [S_CH, R_CH], FP32, kind="ExternalInput")
    wv = nc.dram_tensor("wv", [128, 3], FP32, kind="ExternalInput")
    wh = nc.dram_tensor("wh", [128, 3], FP32, kind="ExternalInput")
    w4s = nc.dram_tensor("w4s", [128, 128], MM_DT, kind="ExternalInput")
    bias = nc.dram_tensor("bias", [128, 1], FP32, kind="ExternalInput")
    y = nc.dram_tensor("y", [N_PER_CORE, T_CH, H_IMG, W_IMG], FP32,
                       kind="ExternalOutput")

    with TileContext(nc) as tc:
        with (
            tc.tile_pool(name="consts", bufs=1) as consts,
            tc.tile_pool(name="xin", bufs=2) as xin,
            tc.tile_pool(name="mid", bufs=2) as mid,
            tc.tile_pool(name="oout", bufs=2) as oout,
            tc.tile_pool(name="psA", bufs=2, space="PSUM") as psumA,
            tc.tile_pool(name="psB", bufs=6, space="PSUM") as psumB,
        ):
            w1T_t = consts.tile([S_CH, R_CH], FP32)
            wv_t = consts.tile([128, 3], FP32)
            wh_t = consts.tile([128, 3], FP32)
            w4s_t = consts.tile([128, 128], MM_DT)
            bias_t = consts.tile([128, 1], FP32)
            nc.sync.dma_start(out=w1T_t[:], in_=w1T[:, :])
            nc.sync.dma_start(out=wv_t[:], in_=wv[:, :])
            nc.sync.dma_start(out=wh_t[:], in_=wh[:, :])
            nc.sync.dma_start(out=w4s_t[:], in_=w4s[:, :])
            nc.sync.dma_start(out=bias_t[:], in_=bias[:, :])

            for n in range(N_PER_CORE):
                for s in range(N_STRIPS):
                    h0 = s * HB
                    # ---- load x strip with one halo row each side ----
                    x_t = xin.tile([S_CH, HB + 2, W_IMG], FP32)
                    if s == 0:
                        nc.gpsimd.memset(x_t[:, 0:1, :], 0.0)
                        nc.sync.dma_start(out=x_t[:, 1:HB + 2, :],
                                          in_=x[n, :, 0:HB + 1, :])
                    elif s == N_STRIPS - 1:
                        nc.sync.dma_start(out=x_t[:, 0:HB + 1, :],
                                          in_=x[n, :, h0 - 1:h0 + HB, :])
                        nc.gpsimd.memset(x_t[:, HB + 1:HB + 2, :], 0.0)
                    else:
                        nc.sync.dma_start(out=x_t[:, :, :],
                                          in_=x[n, :, h0 - 1:h0 + HB + 1, :])

                    # ---- stage A: 1x1 S->R, col-tiled x4 ----
                    # h1p[p in grp j, m, :] = h1[row h0 + 8j - 1 + m, :]
                    h1p = mid.tile([128, GB + 2, W_IMG], FP32, tag="h1p")
                    for c in range((GB + 2) // 2):
                        psA = psumA.tile([128, 2, W_IMG], FP32)
                        for j in range(4):
                            r0 = j * GB + 2 * c
                            nc.tensor.matmul(
                                psA[32 * j:32 * j + 32, :, :],
                                w1T_t[:, :],
                                x_t[:, r0:r0 + 2, :],
                                start=True, stop=True,
                                tile_position=(0, 32 * j),
                            )
                        nc.scalar.copy(h1p[:, 2 * c:2 * c + 2, :], psA[:, :, :])

                    # ---- vertical 3x1 depthwise (per-partition scalars) ----
                    h2p = mid.tile([128, GB, W_IMG + 2], FP32, tag="h2p")
                    nc.gpsimd.memset(h2p[:, :, 0:1], 0.0)
                    nc.gpsimd.memset(h2p[:, :, W_IMG + 1:W_IMG + 2], 0.0)
                    h2c = h2p[:, :, 1:W_IMG + 1]
                    nc.vector.tensor_scalar_mul(
                        h2c, h1p[:, 0:GB, :], wv_t[:, 0:1])
                    for kv in (1, 2):
                        nc.vector.scalar_tensor_tensor(
                            h2c, h1p[:, kv:kv + GB, :], wv_t[:, kv:kv + 1], h2c,
                            op0=mybir.AluOpType.mult, op1=mybir.AluOpType.add)

                    # ---- horizontal 1x3 depthwise ----
                    # accumulate taps 0-1 in fp32, round once into the
                    # MM_DT tile on the final tap
                    h3a = mid.tile([128, GB, W_IMG], FP32, tag="h3a")
                    h3 = mid.tile([128, GB, W_IMG], MM_DT, tag="h3")
                    nc.vector.tensor_scalar_mul(
                        h3a[:, :, :], h2p[:, :, 0:W_IMG], wh_t[:, 0:1])
                    nc.vector.scalar_tensor_tensor(
                        h3a[:, :, :], h2p[:, :, 1:1 + W_IMG],
                        wh_t[:, 1:2], h3a[:, :, :],
                        op0=mybir.AluOpType.mult, op1=mybir.AluOpType.add)
                    nc.vector.scalar_tensor_tensor(
                        h3[:, :, :], h2p[:, :, 2:2 + W_IMG],
                        wh_t[:, 2:3], h3a[:, :, :],
                        op0=mybir.AluOpType.mult, op1=mybir.AluOpType.add)

                    # ---- stage B: 1x1 R->T row-tiled x4, + bias ----
                    o_t = oout.tile([T_CH, HB, W_IMG], FP32)
                    for c in range(GB // 2):
                        for g in range(4):
                            psB = psumB.tile([128, 2, W_IMG], FP32)
                            nc.tensor.matmul(
                                psB[:, :, :],
                                w4s_t[32 * g:32 * g + 32, :],
                                h3[32 * g:32 * g + 32, 2 * c:2 * c + 2, :],
                                start=True, stop=True,
                                tile_position=(32 * g, 0),
                            )
                            orow = g * GB + 2 * c
                            # split bias-copies over ACT and DVE to balance
                            if g == 3:
                                nc.vector.tensor_scalar_add(
                                    o_t[:, orow:orow + 2, :], psB[:, :, :],
                                    bias_t[:, 0:1])
                            else:
                                nc.scalar.add(
                                    o_t[:, orow:orow + 2, :], psB[:, :, :],
                                    bias_t[:, 0:1])

                    # output goes out on the scalar HWDGE ring so reads
                    # (sync ring) and writes overlap instead of FIFO-ing
                    # behind each other on one queue
                    nc.scalar.dma_start(out=y[n, :, h0:h0 + HB, :],
                                        in_=o_t[:, :, :])

    _legalize_sync(nc)
    return nc


def _prep_weights(s_to_r_weight, depth_vert_weight, depth_hor_weight,
                  r_to_t_weight, r_to_t_bias):
    w1T = np.ascontiguousarray(
        s_to_r_weight[:, :, 0, 0].T.astype(np.float32))          # [64, 32]
    wv = np.ascontiguousarray(
        np.tile(depth_vert_weight[:, 0, :, 0], (4, 1)).astype(np.float32))
    wh = np.ascontiguousarray(
        np.tile(depth_hor_weight[:, 0, 0, :], (4, 1)).astype(np.float32))
    w4s = np.ascontiguousarray(
        np.tile(r_to_t_weight[:, :, 0, 0].T, (4, 1)).astype(np.float32))
    b = np.ascontiguousarray(
        r_to_t_bias.reshape(T_CH, 1).astype(np.float32))
    return w1T, wv, wh, w4s, b


def kernel(x, s_to_r_weight, depth_vert_weight, depth_hor_weight,
           r_to_t_weight, r_to_t_bias):
    global LAST_EXEC_TIME_NS
    _install_ntff_hook()
    from concourse.bass_utils import run_bass_kernel_spmd

    if "nc" not in _CACHE:
        _CACHE["nc"] = _build_nc()
    nc = _CACHE["nc"]

    x = np.asarray(x, dtype=np.float32)
    w1T, wv, wh, w4s, b = _prep_weights(
        np.asarray(s_to_r_weight), np.asarray(depth_vert_weight),
        np.asarray(depth_hor_weight), np.asarray(r_to_t_weight),
        np.asarray(r_to_t_bias))

    in_maps = []
    for i in range(N_CORES):
        in_maps.append({
            "x": np.ascontiguousarray(x[i * N_PER_CORE:(i + 1) * N_PER_CORE]),
            "w1T": w1T, "wv": wv, "wh": wh, "w4s": w4s, "bias": b,
        })

    trace = bool(int(os.environ.get("KERNEL_TRACE", "0")))
    res = run_bass_kernel_spmd(nc, in_maps, core_ids=list(range(N_CORES)),
                               trace=trace)
    LAST_EXEC_TIME_NS = res.exec_time_ns

    out = np.empty((N_FULL, T_CH, H_IMG, W_IMG), dtype=np.float32)
    for i in range(N_CORES):
        out[i * N_PER_CORE:(i + 1) * N_PER_CORE] = res.results[i]["y"]
    return out
